# revision 114
# baseline (speedup 1.0000x reference)
"""Multi-head attention (RoPE, causal) TRN2 Bass kernel, 8-way sharded.

Problem: B=4, S=1024, D=1024, H=16 heads of dim 64, fp32.
Sharding: batch (4) x head-half (2) -> 8 cores. Each core computes its
batch's attention output for its 8 heads and the partial output
projection (Wo row-block); the host sums the two half-head partials per
batch and adds the (bv @ Wo + bo) constant.

Per-core layout highlights (v2, bf16, 105.3us TimelineSim vs 155.7 baseline):
  - All matmul operands are bf16 (1 cyc/row on the PE at any moving-dim
    size, vs f32r's 4x penalty below 256), accumulating in f32 PSUM;
    inputs/weights ship as bf16, halving DMA traffic; output partials
    are written back bf16 and summed in f32 on the host.
  - Wq/Wk columns are permuted so each 128-row projection chunk holds 4
    heads' even (or odd) RoPE coordinates. Projection PSUM is evicted to
    SBUF bf16 by the scalar engine; RoPE runs on DVE in bf16 (2x mode)
    and band-combines into head-contiguous "comb" tiles ([E(h);O(h)]
    stacked 64 rows per head).
  - Scores use one K=64 matmul per head per 128-key chunk (kcomb
    stationary, qcomb moving), two heads packed per PSUM tile via PE row
    bands; exact causal column trimming (bf16 has no small-N penalty).
  - exp() on ACT straight out of PSUM -> bf16; causality = chunk
    skipping + per-diagonal triangular mask multiply on DVE; the small
    j==3 diagonal chunk rides the j==1 chunk's tile to share one exp
    (attention is ACT-bound, so ACT instruction count matters).
  - V tiles carry 64 ones-columns so the AV matmul emits the softmax
    denominator replicated across 64 PSUM rows; normalization is a
    reciprocal + multiply on DVE (no broadcast matmul needed).
  - Pair-level software pipeline: scores/exp of pair i+1 are emitted
    before the AVs of pair i (deep es pool carries the lag), the V
    projection is interleaved into the first two ACT-bound score blocks
    as PE filler, and qb1's output projection fills qb0's attention.
  - Dummy warm-up matmuls burn the initial DMA wait so the PE p-state
    ramp completes before real work arrives.
"""

import sys

sys.path.insert(0, "/opt/trn_rl_repo")

import ml_dtypes
import numpy as np

import concourse.bass as bass
import concourse.tile as tile
from concourse import bacc, mybir
from concourse.bass_utils import run_bass_kernel_spmd

P = 128
S = 1024
D = 1024
HD = 64
NH_LOCAL = 8  # heads per core
NB = 2  # S halves for projection psum
QB = 2  # q blocks of 512
KC = 8  # k chunks of 128
F32 = mybir.dt.float32
BF16 = mybir.dt.bfloat16
EXP = mybir.ActivationFunctionType.Exp
MULT = mybir.AluOpType.mult
ADD = mybir.AluOpType.add
SUB = mybir.AluOpType.subtract
DIV = mybir.AluOpType.divide

TRACE = False
LAST_RESULTS = None


def _build_core_program(causal=True, zero_bias=False):
    nc = bacc.Bacc(None, target_bir_lowering=False)

    xqT = nc.declare_dram_parameter("xqT", [D, S], BF16, isOutput=False)
    xkT = nc.declare_dram_parameter("xkT", [D, S], BF16, isOutput=False)
    xvT = nc.declare_dram_parameter("xvT", [D, S], BF16, isOutput=False)
    wq = nc.declare_dram_parameter("wq", [D, 512], BF16, isOutput=False)
    wk = nc.declare_dram_parameter("wk", [D, 512], BF16, isOutput=False)
    wv = nc.declare_dram_parameter("wv", [D, 512], BF16, isOutput=False)
    wo = nc.declare_dram_parameter("wo", [512, D], BF16, isOutput=False)
    bqp = nc.declare_dram_parameter("bqp", [P, 4], F32, isOutput=False)
    bkp = nc.declare_dram_parameter("bkp", [P, 4], F32, isOutput=False)
    cosf = nc.declare_dram_parameter("cosf", [P, S], BF16, isOutput=False)
    sinf = nc.declare_dram_parameter("sinf", [P, S], BF16, isOutput=False)
    tri = nc.declare_dram_parameter("tri", [P, P], BF16, isOutput=False)
    outp = nc.declare_dram_parameter("outp", [S, D], BF16, isOutput=True)

    with tile.TileContext(nc) as tc:
        with (
            tc.tile_pool(name="const", bufs=1) as cpool,
            tc.tile_pool(name="xt", bufs=6) as xtpool,
            tc.tile_pool(name="w", bufs=6) as wpool,
            tc.tile_pool(name="eo", bufs=3) as eopool,
            tc.tile_pool(name="tmp", bufs=2) as tmppool,
            tc.tile_pool(name="comb", bufs=1) as combpool,
            tc.tile_pool(name="vsb", bufs=1) as vpool,
            tc.tile_pool(name="es", bufs=24) as espool,
            tc.tile_pool(name="rc", bufs=4) as rcpool,
            tc.tile_pool(name="cp", bufs=8) as cppool,
            tc.tile_pool(name="osb", bufs=4) as opool,
            tc.tile_pool(name="ps2", bufs=3, space="PSUM") as ps2pool,
            tc.tile_pool(name="ps1", bufs=2, space="PSUM") as ps1pool,
        ):
            # ---- persistent tiles ----
            cos_sb = cpool.tile([P, S], BF16, tag="cos")
            sin_sb = cpool.tile([P, S], BF16, tag="sin")
            tri_sb = cpool.tile([P, P], BF16, tag="tri")
            bq_sb = cpool.tile([P, 4], F32, tag="bq")
            bk_sb = cpool.tile([P, 4], F32, tag="bk")
            wo_sb = cpool.tile([P, 4, D], BF16, tag="wo")
            # v_all[:, ki, h, 0:64] = V features; [:, ki, h, 64:128] = 1.0
            # (ones columns make the AV matmul emit the softmax denominator
            # replicated over 64 PSUM rows)
            v_all = vpool.tile([P, KC, NH_LOCAL, 2 * HD], BF16, tag="v")

            def emit_consts():
                nc.sync.dma_start(cos_sb[:], cosf[:])
                nc.sync.dma_start(sin_sb[:], sinf[:])
                if not zero_bias:
                    nc.sync.dma_start(bq_sb[:], bqp[:])
                    nc.sync.dma_start(bk_sb[:], bkp[:])
                nc.gpsimd.memset(v_all[:, :, :, HD : 2 * HD], 1.0)
                if causal:
                    nc.sync.dma_start(tri_sb[:], tri[:])

            # ---- PE p-state warm-up ----
            # The tensor engine ramps to full clock only after ~3us of
            # continuous execution. The first real matmul can't start until
            # its DMAs land (~3.5us), so burn the wait on dummy matmuls over
            # an uninitialized scratch tile (result never read): the ramp
            # completes during the DMA wait and real matmuls run at peak.
            warm_sb = cpool.tile([P, 512], BF16, tag="warm")
            warm_ps = ps2pool.tile([P, 2, 512], F32, tag="ps2", name="warm_ps")
            nc.vector.memset(warm_sb[0:1, 0:P], 0.0)
            for _ in range(24):
                nc.tensor.matmul(
                    warm_ps[:, 0, 0:P], warm_sb[0:1, 0:P], warm_sb[0:1, 0:P],
                    start=True, stop=True,
                )

            # ---- q/k projections + RoPE -> head-contiguous comb tiles ----
            # comb tile for pair p (local heads 2p, 2p+1), rows:
            #   0:32   E(2p)   = e*cos - o*sin
            #   32:64  O(2p)   = e*sin + o*cos
            #   64:96  E(2p+1)
            #   96:128 O(2p+1)
            qcomb = [combpool.tile([P, S], BF16, tag=f"qc{p}", name=f"qc{p}") for p in range(4)]
            kcomb = [combpool.tile([P, S], BF16, tag=f"kc{p}", name=f"kc{p}") for p in range(4)]

            # V tiles allocated up front; their DMAs are interleaved into
            # K's DMA stream (timing only -- semaphores guard the data) so
            # the first V chunks land early enough to feed the PE fillers
            # inside the ACT-bound score blocks
            w_v = [
                wpool.tile([P, KC // 2, 512], BF16, tag="w", name=f"w_v{hf}")
                for hf in range(2)
            ]
            xt_v = [
                xtpool.tile([P, KC // 2, S], BF16, tag="xt", name=f"xt_v{hf}")
                for hf in range(2)
            ]
            v_dma_done = set()

            def emit_v_dma(ks):
                if ks in v_dma_done:
                    return
                v_dma_done.add(ks)
                hf, kl = divmod(ks, KC // 2)
                nc.sync.dma_start(
                    w_v[hf][:, kl, :], wv[ks * P : (ks + 1) * P, :]
                )
                nc.sync.dma_start(
                    xt_v[hf][:, kl, :], xvT[ks * P : (ks + 1) * P, :]
                )

            first = True
            for name, xT, w, b_sb, comb in (
                ("q", xqT, wq, bq_sb, qcomb),
                ("k", xkT, wk, bk_sb, kcomb),
            ):
                xt_h = []
                w_h = []
                for hf in range(2):
                    w_sb = wpool.tile(
                        [P, KC // 2, 512], BF16, tag="w", name=f"w_{name}{hf}"
                    )
                    xt_sb = xtpool.tile(
                        [P, KC // 2, S], BF16, tag="xt", name=f"xt_{name}{hf}"
                    )
                    # single-chunk transfers, w then x per chunk, so the first
                    # matmuls start as early as possible
                    for ks in range(KC // 2):
                        kg = hf * (KC // 2) + ks
                        nc.sync.dma_start(
                            w_sb[:, ks, :], w[kg * P : (kg + 1) * P, :]
                        )
                        nc.sync.dma_start(
                            xt_sb[:, ks, :], xT[kg * P : (kg + 1) * P, :]
                        )
                    w_h.append(w_sb)
                    xt_h.append(xt_sb)
                    if name == "k" and hf == 0:
                        emit_v_dma(0)
                        emit_v_dma(1)
                if first:
                    emit_consts()
                first = False
                for G in range(2):
                    ce, co = 2 * G, 2 * G + 1  # even/odd chunk col indices
                    ps = {}
                    for nb in range(NB):
                        ps[nb] = ps2pool.tile(
                            [P, 2, 512], F32, tag="ps2", name=f"ps_{name}{G}{nb}"
                        )
                    # chunk-ordered accumulation so the PE starts as soon as
                    # the first weight/activation chunks land
                    for ks in range(KC):
                        hf, kl = divmod(ks, KC // 2)
                        for nb in range(NB):
                            sl = slice(nb * 512, (nb + 1) * 512)
                            for eo, c in ((0, ce), (1, co)):
                                nc.tensor.matmul(
                                    ps[nb][:, eo, :],
                                    w_h[hf][:, kl, c * P : (c + 1) * P],
                                    xt_h[hf][:, kl, sl],
                                    start=(ks == 0),
                                    stop=(ks == KC - 1),
                                )
                    for nb in range(NB):
                        sl = slice(nb * 512, (nb + 1) * 512)
                        eo_sb = eopool.tile(
                            [P, 2, 512], BF16, tag="eo", name=f"eo_{name}{G}{nb}"
                        )
                        nc.scalar.copy(eo_sb[:], ps[nb][:])
                        # RoPE: E = (e+be)c - (o+bo)s ; O = (e+be)s + (o+bo)c
                        t_ec = tmppool.tile([P, 512], BF16, tag="t1")
                        t_os = tmppool.tile([P, 512], BF16, tag="t2")
                        t_es = tmppool.tile([P, 512], BF16, tag="t3")
                        t_oc = tmppool.tile([P, 512], BF16, tag="t4")
                        if zero_bias:
                            # TensorTensor runs in the 2x bf16 DVE mode;
                            # TensorScalarPtr does not
                            nc.vector.tensor_tensor(
                                t_ec[:], eo_sb[:, 0, :], cos_sb[:, sl], MULT
                            )
                            nc.vector.tensor_tensor(
                                t_os[:], eo_sb[:, 1, :], sin_sb[:, sl], MULT
                            )
                            nc.vector.tensor_tensor(
                                t_es[:], eo_sb[:, 0, :], sin_sb[:, sl], MULT
                            )
                            nc.vector.tensor_tensor(
                                t_oc[:], eo_sb[:, 1, :], cos_sb[:, sl], MULT
                            )
                        else:
                            nc.vector.scalar_tensor_tensor(
                                t_ec[:], eo_sb[:, 0, :], b_sb[:, ce : ce + 1],
                                cos_sb[:, sl], ADD, MULT,
                            )
                            nc.vector.scalar_tensor_tensor(
                                t_os[:], eo_sb[:, 1, :], b_sb[:, co : co + 1],
                                sin_sb[:, sl], ADD, MULT,
                            )
                            nc.vector.scalar_tensor_tensor(
                                t_es[:], eo_sb[:, 0, :], b_sb[:, ce : ce + 1],
                                sin_sb[:, sl], ADD, MULT,
                            )
                            nc.vector.scalar_tensor_tensor(
                                t_oc[:], eo_sb[:, 1, :], b_sb[:, co : co + 1],
                                cos_sb[:, sl], ADD, MULT,
                            )
                        # band-wise combine into head-contiguous comb tiles
                        for i in range(4):
                            p = 2 * G + i // 2
                            base = 64 * (i % 2)
                            bs = slice(32 * i, 32 * i + 32)
                            nc.vector.tensor_tensor(
                                comb[p][base : base + 32, sl],
                                t_ec[bs, :], t_os[bs, :], SUB,
                            )
                            nc.vector.tensor_tensor(
                                comb[p][base + 32 : base + 64, sl],
                                t_es[bs, :], t_oc[bs, :], ADD,
                            )

            # ---- remaining V DMAs (chunks 0-1 were interleaved into K's
            # stream above) ----
            for ks in range(KC):
                emit_v_dma(ks)
            for p4 in range(4):
                nc.sync.dma_start(wo_sb[:, p4, :], wo[p4 * P : (p4 + 1) * P, :])

            v_queue = list(range(KC))

            def v_one():
                # one V seq-chunk projection; popped between score chunks so
                # its matmuls sit at interleaved priorities (the 4-deep engine
                # wait queue can't look past a blocked instruction)
                if not v_queue:
                    return
                ki = v_queue.pop(0)
                pool_v = ps1pool if ki % 2 == 0 else ps2pool
                tag_v = "ps1" if ki % 2 == 0 else "ps2"
                ps_v = pool_v.tile([P, 512], F32, tag=tag_v, name=f"psv{ki}")
                for ks in range(KC):
                    hf, kl = divmod(ks, KC // 2)
                    nc.tensor.matmul(
                        ps_v[:],
                        xt_v[hf][:, kl, ki * P : (ki + 1) * P],
                        w_v[hf][:, kl, :],
                        start=(ks == 0),
                        stop=(ks == KC - 1),
                    )
                nc.scalar.copy(
                    v_all[:, ki, :, 0:HD],
                    ps_v[:].rearrange("p (h d) -> p h d", h=NH_LOCAL),
                )

            def v_proj_block(k0=0, k1=KC):
                for _ in range(k0, k1):
                    v_one()

            # ---- attention (pair-level software pipeline) ----
            # scores/exp/tri of pair i+1 are emitted BEFORE the AVs of pair i,
            # so the AV chain never waits on a same-pair exp; the deep es pool
            # carries exp results across the one-pair lag. qb1 (the long half)
            # runs first and its out-projection is emitted right after its
            # last AV block so it fills qb0's ACT-bound attention.
            pair_list = [(1, p) for p in range(4)] + [(0, p) for p in range(4)]
            plan = {}  # (qb,p) -> (order, [(ki, aq0, at0, es_tile), ...])
            cpt = {}  # (pair, qb) -> normalized ctx [128 = 2 heads x 64f, 512q]

            ofill_queue = []

            def ofill_one():
                if ofill_queue:
                    ofill_queue.pop(0)()

            def scores_block(qb, p, vfill=False, ofill=False):
                # The diagonal j==3 chunk (128 live q cols) writes its scores
                # into the unused cols 0:128 of the j==1 chunk's tile, sharing
                # one exp() (attention is ACT-bound, so fewer ACT instrs).
                if causal:
                    order = [0, 1, 3, 2] if qb == 0 else [0, 1, 2, 3, 4, 5, 7, 6]
                else:
                    order = list(range(KC))
                sc2_m = es2_m = None
                recs = []
                for ki in order:
                    ksl = slice(ki * P, (ki + 1) * P)
                    j = ki - 4 * qb if causal else -1
                    q0 = max(0, 128 * j)  # first live q col in this block
                    merged = causal and j == 3  # rides the j==1 tile
                    if merged:
                        sc_t, es_t, t0 = sc2_m, es2_m, 0
                    else:
                        sc_t = ps2pool.tile(
                            [P, 2, 512], F32, tag="ps2", name=f"sc_{qb}_{p}_{ki}"
                        )
                        es_t = espool.tile([P, 2, 512], BF16, tag="es")
                        t0 = q0
                    for ii in range(2):
                        nc.tensor.matmul(
                            sc_t[:, ii, t0 : t0 + 512 - q0],
                            kcomb[p][64 * ii : 64 * ii + 64, ksl],
                            qcomb[p][
                                64 * ii : 64 * ii + 64,
                                qb * 512 + q0 : (qb + 1) * 512,
                            ],
                            start=True,
                            stop=True,
                        )
                    if causal and j == 1:
                        sc2_m, es2_m = sc_t, es_t  # exp deferred until j==3
                        recs.append((ki, q0, 128, es_t))
                        continue
                    if merged:
                        nc.scalar.activation(es_t[:, :, :], sc_t[:, :, :], EXP)
                        # mask both diagonal blocks: j==3's at cols 0:128,
                        # j==1's at cols 128:256 (same tri pattern)
                        nc.vector.tensor_tensor(
                            es_t[:, :, 0:256].rearrange("p a (b c) -> p a b c", c=P),
                            es_t[:, :, 0:256].rearrange("p a (b c) -> p a b c", c=P),
                            tri_sb[:, None, None, :].to_broadcast((P, 2, 2, P)),
                            MULT,
                        )
                        recs.append((ki, 384, 0, es_t))
                    else:
                        nc.scalar.activation(es_t[:, :, q0:], sc_t[:, :, q0:], EXP)
                        if j >= 0:
                            nc.vector.tensor_tensor(
                                es_t[:, :, 128 * j : 128 * (j + 1)],
                                es_t[:, :, 128 * j : 128 * (j + 1)],
                                tri_sb[:, None, :].to_broadcast((P, 2, P)),
                                MULT,
                            )
                        recs.append((ki, q0, t0, es_t))
                    if vfill and ki % 2 == 1:
                        v_one()
                    if ofill:
                        ofill_one()
                plan[(qb, p)] = (order, recs)

            def avs_block(qb, p, tail=False):
                order, recs = plan[(qb, p)]
                ctx_t = [
                    ps1pool.tile([P, 512], F32, tag="ps1", name=f"cx_{qb}_{p}_{ii}")
                    for ii in range(2)
                ]
                # head-major AV order: head0's accumulation (and its evict
                # chain) completes while head1's AVs still run
                for ii in range(2):
                    h = 2 * p + ii
                    for aki, aq0, at0, es_t in recs:
                        nc.tensor.matmul(
                            ctx_t[ii][:, aq0:],
                            v_all[:, aki, h, :],
                            es_t[:, ii, at0 : at0 + 512 - aq0],
                            start=(aki == order[0]),
                            stop=(aki == order[-1]),
                        )
                # normalize + evict; per-head recip->mult. Both recips write
                # the SAME rc rows: the WAR dependency forces the greedy
                # scheduler to run mult0 before recip1, so ctx slot 0 frees
                # after 2 DVE ops instead of 3.
                rc = rcpool.tile([P, 512], BF16, tag="rc")
                cp = cppool.tile([P, 512], BF16, tag="cp", name=f"cp_{qb}_{p}")
                cpt[(p, qb)] = cp
                if tail:
                    # endgame pairs: ACT is idle by now -- evict ctx to SBUF
                    # on ACT (parallel with the DVE recip) so the multiply
                    # runs on fast bf16 SBUF and the PSUM slot frees early
                    cse = espool.tile([P, 2, 512], BF16, tag="es", name=f"cse_{qb}_{p}")
                    for ii in range(2):
                        nc.scalar.copy(cse[:, ii, :], ctx_t[ii][:])
                for ii in range(2):
                    with nc.allow_low_precision(
                        reason="softmax denom reciprocal in bf16 (~4e-3 rel)"
                    ):
                        nc.vector.reciprocal(rc[0:HD, :], ctx_t[ii][HD : 2 * HD, :])
                    nc.vector.tensor_tensor(
                        cp[64 * ii : 64 * ii + 64, :],
                        cse[0:HD, ii, :] if tail else ctx_t[ii][0:HD, :],
                        rc[0:HD, :],
                        MULT,
                    )

            def oproj_block(qb, qi0=0, qi1=4, defer=False):
                for qi in range(qi0, qi1):
                    if defer:
                        ofill_queue.append(
                            lambda qb=qb, qi=qi: oproj_emit(qb, qi)
                        )
                        continue
                    oproj_emit(qb, qi)

            def oproj_emit(qb, qi):
                if True:
                    o_sb = opool.tile([P, D], BF16, tag="o")
                    for dh in range(2):
                        # alternate psum pools so out-proj doesn't serialize
                        # behind the ctx-slot evict chain
                        pool = ps2pool if dh == 0 else ps1pool
                        tag = "ps2" if dh == 0 else "ps1"
                        ps_o = pool.tile(
                            [P, 512], F32, tag=tag, name=f"po_{qb}_{qi}_{dh}"
                        )
                        for pidx in range(4):
                            nc.tensor.matmul(
                                ps_o[:],
                                cpt[(pidx, qb)][:, qi * P : (qi + 1) * P],
                                wo_sb[:, pidx, dh * 512 : (dh + 1) * 512],
                                start=(pidx == 0),
                                stop=(pidx == 3),
                            )
                        # alternate engines so the final evicts drain in
                        # parallel instead of serializing on ACT
                        if dh == 0:
                            nc.scalar.copy(o_sb[:, 0:512], ps_o[:])
                        else:
                            nc.vector.tensor_copy(o_sb[:, 512:D], ps_o[:])
                    # one row-contiguous DMA per 128-row block (half the
                    # HWDGE descriptor-generation slots on the tail); the very
                    # last block DMAs its ACT-evicted half early
                    q0r = (qb * 4 + qi) * P
                    if qb == 0 and qi == 3:
                        nc.sync.dma_start(outp[q0r : q0r + P, 0:512], o_sb[:, 0:512])
                        nc.sync.dma_start(outp[q0r : q0r + P, 512:D], o_sb[:, 512:D])
                    else:
                        nc.sync.dma_start(outp[q0r : q0r + P, :], o_sb[:])

            # First two score blocks run ACT-bound; the V projection emitted
            # after them fills the PE with its matmuls during that window.
            scores_block(1, 0, vfill=True)
            scores_block(1, 1, vfill=True)
            v_proj_block()  # drain any V chunks not consumed as filler
            for i, (qb, p) in enumerate(pair_list):
                if i >= 2:
                    scores_block(qb, p)
                if i > 0:
                    avs_block(*pair_list[i - 1], tail=(pair_list[i - 1] == (0, 2)))
                    if pair_list[i - 1] == (1, 3):
                        oproj_block(1, 0, 1)
                    elif pair_list[i - 1] == (0, 0):
                        oproj_block(1, 1, 2)
                    elif pair_list[i - 1] == (0, 1):
                        oproj_block(1, 2, 3)
                    elif pair_list[i - 1] == (0, 2):
                        oproj_block(1, 3, 4)
            avs_block(*pair_list[-1], tail=True)
            while ofill_queue:
                ofill_one()
            oproj_block(0)

    nc.compile()
    return nc


def _host_prep(query, key, value, Wq, bq, Wk, bk, Wv, bv, Wo, bo):
    """Build the 8 per-core input maps + the shared host-side constants."""
    bf16 = ml_dtypes.bfloat16

    # RoPE tables (matches reference._rope_tables)
    inv_freq = (
        1.0 / (10000.0 ** (np.arange(0, HD, 2, dtype=np.float32) / HD))
    ).astype(np.float32)
    pos = np.arange(S, dtype=np.float32)
    ang = pos[:, None] * inv_freq[None, :]  # [S, 32]
    cos_t = np.cos(ang).astype(np.float32)  # [S, 32]
    sin_t = np.sin(ang).astype(np.float32)
    cosf = np.tile(cos_t.T, (4, 1)).astype(bf16)  # [128, S]
    sinf = np.tile(sin_t.T, (4, 1)).astype(bf16)

    tri = np.triu(np.ones((P, P), dtype=np.float32)).astype(bf16)  # keep kk <= qq

    in_maps = []
    for c in range(8):
        b, g = c // 2, c % 2
        perm = np.concatenate(
            [
                (g * 8 + G * 4 + i) * HD + eo + 2 * np.arange(32)
                for G in range(2)
                for eo in range(2)
                for i in range(4)
            ]
        )
        wq_c = (Wq[:, perm] / 8.0).astype(bf16)
        bq_c = (bq[perm] / 8.0).astype(np.float32).reshape(4, P).T.copy()
        wk_c = Wk[:, perm].astype(bf16)
        bk_c = bk[perm].astype(np.float32).reshape(4, P).T.copy()
        wv_c = Wv[:, g * 512 : (g + 1) * 512].astype(bf16)
        wo_c = Wo[g * 512 : (g + 1) * 512, :].astype(bf16)
        in_maps.append(
            {
                "xqT": np.ascontiguousarray(query[b].T).astype(bf16),
                "xkT": np.ascontiguousarray(key[b].T).astype(bf16),
                "xvT": np.ascontiguousarray(value[b].T).astype(bf16),
                "wq": np.ascontiguousarray(wq_c),
                "wk": np.ascontiguousarray(wk_c),
                "wv": np.ascontiguousarray(wv_c),
                "wo": np.ascontiguousarray(wo_c),
                "bqp": bq_c,
                "bkp": bk_c,
                "cosf": cosf,
                "sinf": sinf,
                "tri": tri,
            }
        )
    extra = (bv.astype(np.float32) @ Wo.astype(np.float32) + bo).astype(np.float32)
    return in_maps, extra


_CACHED = {}


def kernel(query, key, value, mask, Wq, bq, Wk, bk, Wv, bv, Wo, bo):
    global LAST_RESULTS
    query = np.asarray(query, dtype=np.float32)
    key = np.asarray(key, dtype=np.float32)
    value = np.asarray(value, dtype=np.float32)
    Wq, bq = np.asarray(Wq, np.float32), np.asarray(bq, np.float32)
    Wk, bk = np.asarray(Wk, np.float32), np.asarray(bk, np.float32)
    Wv, bv = np.asarray(Wv, np.float32), np.asarray(bv, np.float32)
    Wo, bo = np.asarray(Wo, np.float32), np.asarray(bo, np.float32)

    assert query.shape == (4, S, D), f"kernel hardcodes B=4,S=1024,D=1024, got {query.shape}"
    m2 = np.asarray(mask).reshape(S, S)
    tril = np.tril(np.ones((S, S), m2.dtype))
    if np.array_equal(m2, tril):
        causal = True
    elif np.array_equal(m2, np.ones((S, S), m2.dtype)):
        causal = False
    else:
        raise NotImplementedError("kernel supports causal (tril) or all-ones masks")

    in_maps, extra = _host_prep(
        query, key, value, Wq, bq, Wk, bk, Wv, bv, Wo, bo
    )
    zero_bias = bool((bq == 0).all() and (bk == 0).all())
    key_ = (causal, zero_bias)
    if key_ not in _CACHED:
        _CACHED[key_] = _build_core_program(causal, zero_bias)
    res = run_bass_kernel_spmd(_CACHED[key_], in_maps, list(range(8)), trace=TRACE)
    LAST_RESULTS = res

    B = query.shape[0]
    out = np.empty((B, S, D), dtype=np.float32)
    for b in range(B):
        out[b] = (
            res.results[2 * b]["outp"].astype(np.float32)
            + res.results[2 * b + 1]["outp"].astype(np.float32)
            + extra
        )
    return out


# revision 115
# speedup vs baseline: 1.0008x; 1.0008x over previous
"""Multi-head attention (RoPE, causal) TRN2 Bass kernel, 8-way sharded.

Problem: B=4, S=1024, D=1024, H=16 heads of dim 64, fp32.
Sharding: batch (4) x head-half (2) -> 8 cores. Each core computes its
batch's attention output for its 8 heads and the partial output
projection (Wo row-block); the host sums the two half-head partials per
batch and adds the (bv @ Wo + bo) constant.

Per-core layout highlights (v2, bf16, 105.3us TimelineSim vs 155.7 baseline):
  - All matmul operands are bf16 (1 cyc/row on the PE at any moving-dim
    size, vs f32r's 4x penalty below 256), accumulating in f32 PSUM;
    inputs/weights ship as bf16, halving DMA traffic; output partials
    are written back bf16 and summed in f32 on the host.
  - Wq/Wk columns are permuted so each 128-row projection chunk holds 4
    heads' even (or odd) RoPE coordinates. Projection PSUM is evicted to
    SBUF bf16 by the scalar engine; RoPE runs on DVE in bf16 (2x mode)
    and band-combines into head-contiguous "comb" tiles ([E(h);O(h)]
    stacked 64 rows per head).
  - Scores use one K=64 matmul per head per 128-key chunk (kcomb
    stationary, qcomb moving), two heads packed per PSUM tile via PE row
    bands; exact causal column trimming (bf16 has no small-N penalty).
  - exp() on ACT straight out of PSUM -> bf16; causality = chunk
    skipping + per-diagonal triangular mask multiply on DVE; the small
    j==3 diagonal chunk rides the j==1 chunk's tile to share one exp
    (attention is ACT-bound, so ACT instruction count matters).
  - V tiles carry 64 ones-columns so the AV matmul emits the softmax
    denominator replicated across 64 PSUM rows; normalization is a
    reciprocal + multiply on DVE (no broadcast matmul needed).
  - Pair-level software pipeline: scores/exp of pair i+1 are emitted
    before the AVs of pair i (deep es pool carries the lag), the V
    projection is interleaved into the first two ACT-bound score blocks
    as PE filler, and qb1's output projection fills qb0's attention.
  - Dummy warm-up matmuls burn the initial DMA wait so the PE p-state
    ramp completes before real work arrives.
"""

import sys

sys.path.insert(0, "/opt/trn_rl_repo")

import ml_dtypes
import numpy as np

import concourse.bass as bass
import concourse.tile as tile
from concourse import bacc, mybir
from concourse.bass_utils import run_bass_kernel_spmd

P = 128
S = 1024
D = 1024
HD = 64
NH_LOCAL = 8  # heads per core
NB = 2  # S halves for projection psum
QB = 2  # q blocks of 512
KC = 8  # k chunks of 128
F32 = mybir.dt.float32
BF16 = mybir.dt.bfloat16
EXP = mybir.ActivationFunctionType.Exp
MULT = mybir.AluOpType.mult
ADD = mybir.AluOpType.add
SUB = mybir.AluOpType.subtract
DIV = mybir.AluOpType.divide

TRACE = False
LAST_RESULTS = None


def _build_core_program(causal=True, zero_bias=False):
    nc = bacc.Bacc(None, target_bir_lowering=False)

    xqT = nc.declare_dram_parameter("xqT", [D, S], BF16, isOutput=False)
    xkT = nc.declare_dram_parameter("xkT", [D, S], BF16, isOutput=False)
    xvT = nc.declare_dram_parameter("xvT", [D, S], BF16, isOutput=False)
    wq = nc.declare_dram_parameter("wq", [D, 512], BF16, isOutput=False)
    wk = nc.declare_dram_parameter("wk", [D, 512], BF16, isOutput=False)
    wv = nc.declare_dram_parameter("wv", [D, 512], BF16, isOutput=False)
    wo = nc.declare_dram_parameter("wo", [512, D], BF16, isOutput=False)
    bqp = nc.declare_dram_parameter("bqp", [P, 4], F32, isOutput=False)
    bkp = nc.declare_dram_parameter("bkp", [P, 4], F32, isOutput=False)
    cosf = nc.declare_dram_parameter("cosf", [P, S], BF16, isOutput=False)
    sinf = nc.declare_dram_parameter("sinf", [P, S], BF16, isOutput=False)
    tri = nc.declare_dram_parameter("tri", [P, P], BF16, isOutput=False)
    outp = nc.declare_dram_parameter("outp", [S, D], BF16, isOutput=True)

    with tile.TileContext(nc) as tc:
        with (
            tc.tile_pool(name="const", bufs=1) as cpool,
            tc.tile_pool(name="xt", bufs=6) as xtpool,
            tc.tile_pool(name="w", bufs=6) as wpool,
            tc.tile_pool(name="eo", bufs=3) as eopool,
            tc.tile_pool(name="tmp", bufs=2) as tmppool,
            tc.tile_pool(name="comb", bufs=1) as combpool,
            tc.tile_pool(name="vsb", bufs=1) as vpool,
            tc.tile_pool(name="es", bufs=24) as espool,
            tc.tile_pool(name="rc", bufs=4) as rcpool,
            tc.tile_pool(name="cp", bufs=8) as cppool,
            tc.tile_pool(name="osb", bufs=6) as opool,
            tc.tile_pool(name="ps2", bufs=3, space="PSUM") as ps2pool,
            tc.tile_pool(name="ps1", bufs=2, space="PSUM") as ps1pool,
        ):
            # ---- persistent tiles ----
            cos_sb = cpool.tile([P, S], BF16, tag="cos")
            sin_sb = cpool.tile([P, S], BF16, tag="sin")
            tri_sb = cpool.tile([P, P], BF16, tag="tri")
            bq_sb = cpool.tile([P, 4], F32, tag="bq")
            bk_sb = cpool.tile([P, 4], F32, tag="bk")
            wo_sb = cpool.tile([P, 4, D], BF16, tag="wo")
            # v_all[:, ki, h, 0:64] = V features; [:, ki, h, 64:128] = 1.0
            # (ones columns make the AV matmul emit the softmax denominator
            # replicated over 64 PSUM rows)
            v_all = vpool.tile([P, KC, NH_LOCAL, 2 * HD], BF16, tag="v")

            def emit_consts():
                nc.sync.dma_start(cos_sb[:], cosf[:])
                nc.sync.dma_start(sin_sb[:], sinf[:])
                if not zero_bias:
                    nc.sync.dma_start(bq_sb[:], bqp[:])
                    nc.sync.dma_start(bk_sb[:], bkp[:])
                nc.gpsimd.memset(v_all[:, :, :, HD : 2 * HD], 1.0)
                if causal:
                    nc.sync.dma_start(tri_sb[:], tri[:])

            # ---- PE p-state warm-up ----
            # The tensor engine ramps to full clock only after ~3us of
            # continuous execution. The first real matmul can't start until
            # its DMAs land (~3.5us), so burn the wait on dummy matmuls over
            # an uninitialized scratch tile (result never read): the ramp
            # completes during the DMA wait and real matmuls run at peak.
            warm_sb = cpool.tile([P, 512], BF16, tag="warm")
            warm_ps = ps2pool.tile([P, 2, 512], F32, tag="ps2", name="warm_ps")
            nc.vector.memset(warm_sb[0:1, 0:P], 0.0)
            for _ in range(24):
                nc.tensor.matmul(
                    warm_ps[:, 0, 0:P], warm_sb[0:1, 0:P], warm_sb[0:1, 0:P],
                    start=True, stop=True,
                )

            # ---- q/k projections + RoPE -> head-contiguous comb tiles ----
            # comb tile for pair p (local heads 2p, 2p+1), rows:
            #   0:32   E(2p)   = e*cos - o*sin
            #   32:64  O(2p)   = e*sin + o*cos
            #   64:96  E(2p+1)
            #   96:128 O(2p+1)
            qcomb = [combpool.tile([P, S], BF16, tag=f"qc{p}", name=f"qc{p}") for p in range(4)]
            kcomb = [combpool.tile([P, S], BF16, tag=f"kc{p}", name=f"kc{p}") for p in range(4)]

            # V tiles allocated up front; their DMAs are interleaved into
            # K's DMA stream (timing only -- semaphores guard the data) so
            # the first V chunks land early enough to feed the PE fillers
            # inside the ACT-bound score blocks
            w_v = [
                wpool.tile([P, KC // 2, 512], BF16, tag="w", name=f"w_v{hf}")
                for hf in range(2)
            ]
            xt_v = [
                xtpool.tile([P, KC // 2, S], BF16, tag="xt", name=f"xt_v{hf}")
                for hf in range(2)
            ]
            v_dma_done = set()

            def emit_v_dma(ks):
                if ks in v_dma_done:
                    return
                v_dma_done.add(ks)
                hf, kl = divmod(ks, KC // 2)
                nc.sync.dma_start(
                    w_v[hf][:, kl, :], wv[ks * P : (ks + 1) * P, :]
                )
                nc.sync.dma_start(
                    xt_v[hf][:, kl, :], xvT[ks * P : (ks + 1) * P, :]
                )

            first = True
            for name, xT, w, b_sb, comb in (
                ("q", xqT, wq, bq_sb, qcomb),
                ("k", xkT, wk, bk_sb, kcomb),
            ):
                xt_h = []
                w_h = []
                for hf in range(2):
                    w_sb = wpool.tile(
                        [P, KC // 2, 512], BF16, tag="w", name=f"w_{name}{hf}"
                    )
                    xt_sb = xtpool.tile(
                        [P, KC // 2, S], BF16, tag="xt", name=f"xt_{name}{hf}"
                    )
                    # single-chunk transfers, w then x per chunk, so the first
                    # matmuls start as early as possible
                    for ks in range(KC // 2):
                        kg = hf * (KC // 2) + ks
                        nc.sync.dma_start(
                            w_sb[:, ks, :], w[kg * P : (kg + 1) * P, :]
                        )
                        nc.sync.dma_start(
                            xt_sb[:, ks, :], xT[kg * P : (kg + 1) * P, :]
                        )
                    w_h.append(w_sb)
                    xt_h.append(xt_sb)
                    if name == "k" and hf == 0:
                        emit_v_dma(0)
                        emit_v_dma(1)
                if first:
                    emit_consts()
                first = False
                for G in range(2):
                    ce, co = 2 * G, 2 * G + 1  # even/odd chunk col indices
                    ps = {}
                    for nb in range(NB):
                        ps[nb] = ps2pool.tile(
                            [P, 2, 512], F32, tag="ps2", name=f"ps_{name}{G}{nb}"
                        )
                    # chunk-ordered accumulation so the PE starts as soon as
                    # the first weight/activation chunks land
                    for ks in range(KC):
                        hf, kl = divmod(ks, KC // 2)
                        for nb in range(NB):
                            sl = slice(nb * 512, (nb + 1) * 512)
                            for eo, c in ((0, ce), (1, co)):
                                nc.tensor.matmul(
                                    ps[nb][:, eo, :],
                                    w_h[hf][:, kl, c * P : (c + 1) * P],
                                    xt_h[hf][:, kl, sl],
                                    start=(ks == 0),
                                    stop=(ks == KC - 1),
                                )
                    for nb in range(NB):
                        sl = slice(nb * 512, (nb + 1) * 512)
                        eo_sb = eopool.tile(
                            [P, 2, 512], BF16, tag="eo", name=f"eo_{name}{G}{nb}"
                        )
                        nc.scalar.copy(eo_sb[:], ps[nb][:])
                        # RoPE: E = (e+be)c - (o+bo)s ; O = (e+be)s + (o+bo)c
                        t_ec = tmppool.tile([P, 512], BF16, tag="t1")
                        t_os = tmppool.tile([P, 512], BF16, tag="t2")
                        t_es = tmppool.tile([P, 512], BF16, tag="t3")
                        t_oc = tmppool.tile([P, 512], BF16, tag="t4")
                        if zero_bias:
                            # TensorTensor runs in the 2x bf16 DVE mode;
                            # TensorScalarPtr does not
                            nc.vector.tensor_tensor(
                                t_ec[:], eo_sb[:, 0, :], cos_sb[:, sl], MULT
                            )
                            nc.vector.tensor_tensor(
                                t_os[:], eo_sb[:, 1, :], sin_sb[:, sl], MULT
                            )
                            nc.vector.tensor_tensor(
                                t_es[:], eo_sb[:, 0, :], sin_sb[:, sl], MULT
                            )
                            nc.vector.tensor_tensor(
                                t_oc[:], eo_sb[:, 1, :], cos_sb[:, sl], MULT
                            )
                        else:
                            nc.vector.scalar_tensor_tensor(
                                t_ec[:], eo_sb[:, 0, :], b_sb[:, ce : ce + 1],
                                cos_sb[:, sl], ADD, MULT,
                            )
                            nc.vector.scalar_tensor_tensor(
                                t_os[:], eo_sb[:, 1, :], b_sb[:, co : co + 1],
                                sin_sb[:, sl], ADD, MULT,
                            )
                            nc.vector.scalar_tensor_tensor(
                                t_es[:], eo_sb[:, 0, :], b_sb[:, ce : ce + 1],
                                sin_sb[:, sl], ADD, MULT,
                            )
                            nc.vector.scalar_tensor_tensor(
                                t_oc[:], eo_sb[:, 1, :], b_sb[:, co : co + 1],
                                cos_sb[:, sl], ADD, MULT,
                            )
                        # band-wise combine into head-contiguous comb tiles
                        for i in range(4):
                            p = 2 * G + i // 2
                            base = 64 * (i % 2)
                            bs = slice(32 * i, 32 * i + 32)
                            nc.vector.tensor_tensor(
                                comb[p][base : base + 32, sl],
                                t_ec[bs, :], t_os[bs, :], SUB,
                            )
                            nc.vector.tensor_tensor(
                                comb[p][base + 32 : base + 64, sl],
                                t_es[bs, :], t_oc[bs, :], ADD,
                            )

            # ---- remaining V DMAs (chunks 0-1 were interleaved into K's
            # stream above) ----
            for ks in range(KC):
                emit_v_dma(ks)
            for p4 in range(4):
                nc.sync.dma_start(wo_sb[:, p4, :], wo[p4 * P : (p4 + 1) * P, :])

            v_queue = list(range(KC))

            def v_one():
                # one V seq-chunk projection; popped between score chunks so
                # its matmuls sit at interleaved priorities (the 4-deep engine
                # wait queue can't look past a blocked instruction)
                if not v_queue:
                    return
                ki = v_queue.pop(0)
                pool_v = ps1pool if ki % 2 == 0 else ps2pool
                tag_v = "ps1" if ki % 2 == 0 else "ps2"
                ps_v = pool_v.tile([P, 512], F32, tag=tag_v, name=f"psv{ki}")
                for ks in range(KC):
                    hf, kl = divmod(ks, KC // 2)
                    nc.tensor.matmul(
                        ps_v[:],
                        xt_v[hf][:, kl, ki * P : (ki + 1) * P],
                        w_v[hf][:, kl, :],
                        start=(ks == 0),
                        stop=(ks == KC - 1),
                    )
                nc.scalar.copy(
                    v_all[:, ki, :, 0:HD],
                    ps_v[:].rearrange("p (h d) -> p h d", h=NH_LOCAL),
                )

            def v_proj_block(k0=0, k1=KC):
                for _ in range(k0, k1):
                    v_one()

            # ---- attention (pair-level software pipeline) ----
            # scores/exp/tri of pair i+1 are emitted BEFORE the AVs of pair i,
            # so the AV chain never waits on a same-pair exp; the deep es pool
            # carries exp results across the one-pair lag. qb1 (the long half)
            # runs first and its out-projection is emitted right after its
            # last AV block so it fills qb0's ACT-bound attention.
            pair_list = [(1, p) for p in range(4)] + [(0, p) for p in range(4)]
            plan = {}  # (qb,p) -> (order, [(ki, aq0, at0, es_tile), ...])
            cpt = {}  # (pair, qb) -> normalized ctx [128 = 2 heads x 64f, 512q]

            ofill_queue = []

            def ofill_one():
                if ofill_queue:
                    ofill_queue.pop(0)()

            def scores_block(qb, p, vfill=False, ofill=False):
                # The diagonal j==3 chunk (128 live q cols) writes its scores
                # into the unused cols 0:128 of the j==1 chunk's tile, sharing
                # one exp() (attention is ACT-bound, so fewer ACT instrs).
                if causal:
                    order = [0, 1, 3, 2] if qb == 0 else [0, 1, 2, 3, 4, 5, 7, 6]
                else:
                    order = list(range(KC))
                sc2_m = es2_m = None
                recs = []
                for ki in order:
                    ksl = slice(ki * P, (ki + 1) * P)
                    j = ki - 4 * qb if causal else -1
                    q0 = max(0, 128 * j)  # first live q col in this block
                    merged = causal and j == 3  # rides the j==1 tile
                    if merged:
                        sc_t, es_t, t0 = sc2_m, es2_m, 0
                    else:
                        sc_t = ps2pool.tile(
                            [P, 2, 512], F32, tag="ps2", name=f"sc_{qb}_{p}_{ki}"
                        )
                        es_t = espool.tile([P, 2, 512], BF16, tag="es")
                        t0 = q0
                    for ii in range(2):
                        nc.tensor.matmul(
                            sc_t[:, ii, t0 : t0 + 512 - q0],
                            kcomb[p][64 * ii : 64 * ii + 64, ksl],
                            qcomb[p][
                                64 * ii : 64 * ii + 64,
                                qb * 512 + q0 : (qb + 1) * 512,
                            ],
                            start=True,
                            stop=True,
                        )
                    if causal and j == 1:
                        sc2_m, es2_m = sc_t, es_t  # exp deferred until j==3
                        recs.append((ki, q0, 128, es_t))
                        continue
                    if merged:
                        nc.scalar.activation(es_t[:, :, :], sc_t[:, :, :], EXP)
                        # mask both diagonal blocks: j==3's at cols 0:128,
                        # j==1's at cols 128:256 (same tri pattern)
                        nc.vector.tensor_tensor(
                            es_t[:, :, 0:256].rearrange("p a (b c) -> p a b c", c=P),
                            es_t[:, :, 0:256].rearrange("p a (b c) -> p a b c", c=P),
                            tri_sb[:, None, None, :].to_broadcast((P, 2, 2, P)),
                            MULT,
                        )
                        recs.append((ki, 384, 0, es_t))
                    else:
                        nc.scalar.activation(es_t[:, :, q0:], sc_t[:, :, q0:], EXP)
                        if j >= 0:
                            nc.vector.tensor_tensor(
                                es_t[:, :, 128 * j : 128 * (j + 1)],
                                es_t[:, :, 128 * j : 128 * (j + 1)],
                                tri_sb[:, None, :].to_broadcast((P, 2, P)),
                                MULT,
                            )
                        recs.append((ki, q0, t0, es_t))
                    if vfill and ki % 2 == 1:
                        v_one()
                    if ofill:
                        ofill_one()
                plan[(qb, p)] = (order, recs)

            def avs_block(qb, p, tail=False):
                order, recs = plan[(qb, p)]
                ctx_t = [
                    ps1pool.tile([P, 512], F32, tag="ps1", name=f"cx_{qb}_{p}_{ii}")
                    for ii in range(2)
                ]
                # head-major AV order: head0's accumulation (and its evict
                # chain) completes while head1's AVs still run
                for ii in range(2):
                    h = 2 * p + ii
                    for aki, aq0, at0, es_t in recs:
                        nc.tensor.matmul(
                            ctx_t[ii][:, aq0:],
                            v_all[:, aki, h, :],
                            es_t[:, ii, at0 : at0 + 512 - aq0],
                            start=(aki == order[0]),
                            stop=(aki == order[-1]),
                        )
                # normalize + evict; per-head recip->mult. Both recips write
                # the SAME rc rows: the WAR dependency forces the greedy
                # scheduler to run mult0 before recip1, so ctx slot 0 frees
                # after 2 DVE ops instead of 3.
                rc = rcpool.tile([P, 512], BF16, tag="rc")
                cp = cppool.tile([P, 512], BF16, tag="cp", name=f"cp_{qb}_{p}")
                cpt[(p, qb)] = cp
                if tail:
                    # endgame pairs: ACT is idle by now -- evict ctx to SBUF
                    # on ACT (parallel with the DVE recip) so the multiply
                    # runs on fast bf16 SBUF and the PSUM slot frees early
                    cse = espool.tile([P, 2, 512], BF16, tag="es", name=f"cse_{qb}_{p}")
                    for ii in range(2):
                        nc.scalar.copy(cse[:, ii, :], ctx_t[ii][:])
                for ii in range(2):
                    with nc.allow_low_precision(
                        reason="softmax denom reciprocal in bf16 (~4e-3 rel)"
                    ):
                        nc.vector.reciprocal(rc[0:HD, :], ctx_t[ii][HD : 2 * HD, :])
                    nc.vector.tensor_tensor(
                        cp[64 * ii : 64 * ii + 64, :],
                        cse[0:HD, ii, :] if tail else ctx_t[ii][0:HD, :],
                        rc[0:HD, :],
                        MULT,
                    )

            def oproj_block(qb, qi0=0, qi1=4, defer=False):
                for qi in range(qi0, qi1):
                    if defer:
                        ofill_queue.append(
                            lambda qb=qb, qi=qi: oproj_emit(qb, qi)
                        )
                        continue
                    oproj_emit(qb, qi)

            def oproj_emit(qb, qi):
                if True:
                    o_sb = opool.tile([P, D], BF16, tag="o")
                    for dh in range(2):
                        # alternate psum pools so out-proj doesn't serialize
                        # behind the ctx-slot evict chain
                        pool = ps2pool if dh == 0 else ps1pool
                        tag = "ps2" if dh == 0 else "ps1"
                        ps_o = pool.tile(
                            [P, 512], F32, tag=tag, name=f"po_{qb}_{qi}_{dh}"
                        )
                        for pidx in range(4):
                            nc.tensor.matmul(
                                ps_o[:],
                                cpt[(pidx, qb)][:, qi * P : (qi + 1) * P],
                                wo_sb[:, pidx, dh * 512 : (dh + 1) * 512],
                                start=(pidx == 0),
                                stop=(pidx == 3),
                            )
                        # alternate engines so the final evicts drain in
                        # parallel instead of serializing on ACT
                        if dh == 0:
                            nc.scalar.copy(o_sb[:, 0:512], ps_o[:])
                        else:
                            nc.vector.tensor_copy(o_sb[:, 512:D], ps_o[:])
                    # one row-contiguous DMA per 128-row block (half the
                    # HWDGE descriptor-generation slots on the tail); the very
                    # last block DMAs its ACT-evicted half early
                    q0r = (qb * 4 + qi) * P
                    if qb == 0 and qi == 3:
                        nc.sync.dma_start(outp[q0r : q0r + P, 0:512], o_sb[:, 0:512])
                        nc.sync.dma_start(outp[q0r : q0r + P, 512:D], o_sb[:, 512:D])
                    else:
                        nc.sync.dma_start(outp[q0r : q0r + P, :], o_sb[:])

            # First two score blocks run ACT-bound; the V projection emitted
            # after them fills the PE with its matmuls during that window.
            scores_block(1, 0, vfill=True)
            scores_block(1, 1, vfill=True)
            v_proj_block()  # drain any V chunks not consumed as filler
            for i, (qb, p) in enumerate(pair_list):
                if i >= 2:
                    scores_block(qb, p)
                if i > 0:
                    avs_block(*pair_list[i - 1], tail=(pair_list[i - 1] == (0, 2)))
                    if pair_list[i - 1] == (1, 3):
                        oproj_block(1, 0, 1)
                    elif pair_list[i - 1] == (0, 0):
                        oproj_block(1, 1, 2)
                    elif pair_list[i - 1] == (0, 1):
                        oproj_block(1, 2, 3)
                    elif pair_list[i - 1] == (0, 2):
                        oproj_block(1, 3, 4)
            avs_block(*pair_list[-1], tail=True)
            while ofill_queue:
                ofill_one()
            oproj_block(0)

    nc.compile()
    return nc


def _host_prep(query, key, value, Wq, bq, Wk, bk, Wv, bv, Wo, bo):
    """Build the 8 per-core input maps + the shared host-side constants."""
    bf16 = ml_dtypes.bfloat16

    # RoPE tables (matches reference._rope_tables)
    inv_freq = (
        1.0 / (10000.0 ** (np.arange(0, HD, 2, dtype=np.float32) / HD))
    ).astype(np.float32)
    pos = np.arange(S, dtype=np.float32)
    ang = pos[:, None] * inv_freq[None, :]  # [S, 32]
    cos_t = np.cos(ang).astype(np.float32)  # [S, 32]
    sin_t = np.sin(ang).astype(np.float32)
    cosf = np.tile(cos_t.T, (4, 1)).astype(bf16)  # [128, S]
    sinf = np.tile(sin_t.T, (4, 1)).astype(bf16)

    tri = np.triu(np.ones((P, P), dtype=np.float32)).astype(bf16)  # keep kk <= qq

    in_maps = []
    for c in range(8):
        b, g = c // 2, c % 2
        perm = np.concatenate(
            [
                (g * 8 + G * 4 + i) * HD + eo + 2 * np.arange(32)
                for G in range(2)
                for eo in range(2)
                for i in range(4)
            ]
        )
        wq_c = (Wq[:, perm] / 8.0).astype(bf16)
        bq_c = (bq[perm] / 8.0).astype(np.float32).reshape(4, P).T.copy()
        wk_c = Wk[:, perm].astype(bf16)
        bk_c = bk[perm].astype(np.float32).reshape(4, P).T.copy()
        wv_c = Wv[:, g * 512 : (g + 1) * 512].astype(bf16)
        wo_c = Wo[g * 512 : (g + 1) * 512, :].astype(bf16)
        in_maps.append(
            {
                "xqT": np.ascontiguousarray(query[b].T).astype(bf16),
                "xkT": np.ascontiguousarray(key[b].T).astype(bf16),
                "xvT": np.ascontiguousarray(value[b].T).astype(bf16),
                "wq": np.ascontiguousarray(wq_c),
                "wk": np.ascontiguousarray(wk_c),
                "wv": np.ascontiguousarray(wv_c),
                "wo": np.ascontiguousarray(wo_c),
                "bqp": bq_c,
                "bkp": bk_c,
                "cosf": cosf,
                "sinf": sinf,
                "tri": tri,
            }
        )
    extra = (bv.astype(np.float32) @ Wo.astype(np.float32) + bo).astype(np.float32)
    return in_maps, extra


_CACHED = {}


def kernel(query, key, value, mask, Wq, bq, Wk, bk, Wv, bv, Wo, bo):
    global LAST_RESULTS
    query = np.asarray(query, dtype=np.float32)
    key = np.asarray(key, dtype=np.float32)
    value = np.asarray(value, dtype=np.float32)
    Wq, bq = np.asarray(Wq, np.float32), np.asarray(bq, np.float32)
    Wk, bk = np.asarray(Wk, np.float32), np.asarray(bk, np.float32)
    Wv, bv = np.asarray(Wv, np.float32), np.asarray(bv, np.float32)
    Wo, bo = np.asarray(Wo, np.float32), np.asarray(bo, np.float32)

    assert query.shape == (4, S, D), f"kernel hardcodes B=4,S=1024,D=1024, got {query.shape}"
    m2 = np.asarray(mask).reshape(S, S)
    tril = np.tril(np.ones((S, S), m2.dtype))
    if np.array_equal(m2, tril):
        causal = True
    elif np.array_equal(m2, np.ones((S, S), m2.dtype)):
        causal = False
    else:
        raise NotImplementedError("kernel supports causal (tril) or all-ones masks")

    in_maps, extra = _host_prep(
        query, key, value, Wq, bq, Wk, bk, Wv, bv, Wo, bo
    )
    zero_bias = bool((bq == 0).all() and (bk == 0).all())
    key_ = (causal, zero_bias)
    if key_ not in _CACHED:
        _CACHED[key_] = _build_core_program(causal, zero_bias)
    res = run_bass_kernel_spmd(_CACHED[key_], in_maps, list(range(8)), trace=TRACE)
    LAST_RESULTS = res

    B = query.shape[0]
    out = np.empty((B, S, D), dtype=np.float32)
    for b in range(B):
        out[b] = (
            res.results[2 * b]["outp"].astype(np.float32)
            + res.results[2 * b + 1]["outp"].astype(np.float32)
            + extra
        )
    return out
